# revision 1
# baseline (speedup 1.0000x reference)
"""DecoderRNN Trainium2 kernel: 63-step LSTM + Luong attention + vocab projection.

Strategy (8 NeuronCores, SPMD):
  - Recurrence: gates computed TRANSPOSED (gatesT chunks [128, 32]) so LSTM
    elementwise runs on 128 partitions and h is produced directly in hT layout.
    W_hhT tiles (bf16) are the stationary operand, h (bf16) the moving one.
    TP=True: the 4096 gate dims are sharded 8 ways (each core owns 128 hidden
    dims x 4 gates); per-step AllGather of the bf16 h-slice [128, 32].
  - Phase 1 (XgT = W_ih x_t + bias, all steps): sharded with the same gate
    split; stored in DRAM, prefetched per step.
  - Phase 3: attention + W_w decoder replicated on every core (b-sharding would
    need core-dependent static APs, which SPMD forbids); the [H, V] vocab
    projection is sharded by vocab: each core computes logits[:, :, slice(4000)].
  - Host side does layout-only prep: transposes, bf16 casts, embedding row
    gather, per-core weight slicing; output is np.concatenate over the V axis.
"""

import numpy as np
import ml_dtypes
from contextlib import ExitStack

import concourse.bass as bass
import concourse.bacc as bacc
import concourse.tile as tile
import concourse.mybir as mybir
from concourse import masks
from concourse.bass_utils import run_bass_kernel_spmd

F32 = mybir.dt.float32
F32R = mybir.dt.float32r
BF16 = mybir.dt.bfloat16
AF = mybir.ActivationFunctionType

B, T, S = 32, 63, 64          # batch, steps (T-1 of the 64), source len
V, E, H = 32000, 512, 1024
G = 4 * H                     # gate dim
P = 128                       # partitions
NCORES = 8
R = T * B                     # 2016 rows, row index r = t*32 + b
VL = V // NCORES              # 4000 vocab slice per core

TP = True                     # shard the recurrence 8-way with per-step AllGather
HDT_IS_F32R = TP              # h/W_hh/scores datapath dtype (f32r under TP)

KH = H // P                   # 8 k-chunks over hidden
KE = E // P                   # 4 k-chunks over embedding
U = 1 if TP else KH           # hidden-dim chunks owned per core (per gate quarter)
CH = 4 * U                    # gate chunks owned per core
NW = 4                        # stage-A row windows
RW = R // NW                  # 504 rows per window
VN = VL // 500                # 8 vocab n-tiles of 500
TGROUPS = [(4 * i, min(4 * i + 4, T)) for i in range((T + 3) // 4)]  # vocab m-tiles


def build_graph():
    nc = bacc.Bacc("TRN2", target_bir_lowering=False, debug=False,
                   num_devices=NCORES)

    def inp(name, shape, dtype):
        return nc.dram_tensor(name, list(shape), dtype, kind="ExternalInput").ap()

    # --- inputs (per-core data may differ, graph is identical) ---
    x_embT = inp("x_embT", [E, R], BF16)           # embedded tgt, transposed
    w_ihT_s = inp("w_ihT_s", [E, CH * P], BF16)    # cols (q,u,p) for owned chunks
    HDT = F32R if HDT_IS_F32R else BF16
    w_hhT_s = inp("w_hhT_s", [H, CH * P], HDT)
    bias_s = inp("bias_s", [P, CH], F32)           # (b_ih+b_hh) per owned chunk
    h0T = inp("h0T", [H, B], HDT)
    c0T_s = inp("c0T_s", [P, U * B], F32)          # c0 slice, cols (u, b)
    enc = inp("enc", [B, S, H], BF16)              # lhsT for context matmul
    encT = inp("encT", [B, H, S], HDT)             # rhs for scores matmul
    w_wT_h = inp("w_wT_h", [H, H], HDT)            # rows 0:H of W_w.T
    w_wT_c = inp("w_wT_c", [H, H], BF16)           # rows H:2H of W_w.T
    b_w_sb = inp("b_w_sb", [P, KH], F32)
    w_outT_s = inp("w_outT_s", [H, VL], BF16)      # per-core vocab slice
    b_out_s = inp("b_out_s", [1, VL], BF16)
    out_s = nc.dram_tensor("out_s", [B, T, VL], F32, kind="ExternalOutput").ap()

    with tile.TileContext(nc) as tc, ExitStack() as ctx:
        pool1 = ctx.enter_context(tc.tile_pool(name="pool1", bufs=1))
        stream = ctx.enter_context(tc.tile_pool(name="stream", bufs=3))
        work = ctx.enter_context(tc.tile_pool(name="work", bufs=2))
        state = ctx.enter_context(tc.tile_pool(name="state", bufs=2))
        ps_gate = ctx.enter_context(tc.tile_pool(name="ps_gate", bufs=1, space="PSUM"))
        ps_mm = ctx.enter_context(tc.tile_pool(name="ps_mm", bufs=2, space="PSUM"))
        dram = ctx.enter_context(tc.tile_pool(name="dram", bufs=1, space="DRAM"))

        # ---------------- resident tiles ----------------
        hall = [pool1.tile([P, R], HDT, name=f"hall{k}") for k in range(KH)]
        dect = [pool1.tile([P, R], BF16, name=f"dect{k}") for k in range(KH)]
        ctxt = [pool1.tile([P, R], BF16, name=f"ctxt{k}") for k in range(KH)]
        whh = pool1.tile([P, KH, CH * P], HDT, name="whh")
        nc.sync.dma_start(whh[:], w_hhT_s.rearrange("(k p) c -> p k c", p=P))
        wih = pool1.tile([P, KE, CH * P], BF16, name="wih")
        nc.sync.dma_start(wih[:], w_ihT_s.rearrange("(k p) c -> p k c", p=P))
        bias_t = pool1.tile([P, CH], F32, name="bias_t")
        nc.sync.dma_start(bias_t[:], bias_s[:])
        bw_t = pool1.tile([P, KH], F32, name="bw_t")
        nc.sync.dma_start(bw_t[:], b_w_sb[:])
        bout_t = pool1.tile([1, VL], BF16, name="bout_t")
        nc.sync.dma_start(bout_t[:], b_out_s[:])
        ones_t = pool1.tile([1, P], BF16, name="ones_t")
        nc.gpsimd.memset(ones_t[:], 1.0)
        h0_t = pool1.tile([P, KH, B], HDT, name="h0_t")
        nc.sync.dma_start(h0_t[:], h0T.rearrange("(k p) b -> p k b", p=P))
        ident = pool1.tile([P, P], BF16, name="ident")
        masks.make_identity(nc, ident[:])

        xg_dram = dram.tile([CH, P, R], F32, name="xg_dram")
        if TP:
            cc_in = [dram.tile([P, B], HDT, name=f"cc_in{i}") for i in range(T)]
            cc_out = [dram.tile([NCORES * P, B], HDT, name=f"cc_out{i}",
                                addr_space="Shared") for i in range(T)]

        # ---------------- stage A: XgT = W_ihT.T @ x_embT + bias ----------------
        for n in range(NW):
            xtiles = []
            for k in range(KE):
                xt = stream.tile([P, RW], BF16, name="xa", tag=f"xa{k}", bufs=2)
                nc.sync.dma_start(xt[:], x_embT[k * P:(k + 1) * P, n * RW:(n + 1) * RW])
                xtiles.append(xt)
            for c in range(CH):
                ps = ps_mm.tile([P, RW], F32, name="ps_a", tag="psA")
                for k in range(KE):
                    nc.tensor.matmul(
                        ps[:],
                        lhsT=wih[:, k, c * P:(c + 1) * P],
                        rhs=xtiles[k][:],
                        start=(k == 0), stop=(k == KE - 1))
                xg_sb = work.tile([P, RW], F32, name="xg_sb", tag="xg_sb", bufs=1)
                nc.scalar.activation(xg_sb[:], ps[:], AF.Identity,
                                     bias=bias_t[:, c:c + 1])
                nc.sync.dma_start(xg_dram[c, :, n * RW:(n + 1) * RW], xg_sb[:])

        # ---------------- recurrence ----------------
        c0_sb = pool1.tile([P, U * B], F32, name="c0_sb")
        nc.sync.dma_start(c0_sb[:], c0T_s[:])
        c_prev = None
        for t in range(T):
            # gate matmuls: psum[q] [P, U*B] accumulating over KH hidden chunks
            psg = [ps_gate.tile([P, U * B], F32, name=f"psg{q}", tag=f"psg{q}")
                   for q in range(4)]
            for q in range(4):
                for u in range(U):
                    c_idx = q * U + u
                    for k in range(KH):
                        rhs = (h0_t[:, k, :] if t == 0 else
                               hall[k][:, (t - 1) * B: t * B])
                        nc.tensor.matmul(
                            psg[q][:, u * B:(u + 1) * B],
                            lhsT=whh[:, k, c_idx * P:(c_idx + 1) * P],
                            rhs=rhs,
                            start=(k == 0), stop=(k == KH - 1))
            # Xg prefetch for this step: [CH, P, B] window
            xg_t = stream.tile([P, CH, B], F32, name="xg_t", tag="xg_t")
            nc.sync.dma_start(
                xg_t[:],
                xg_dram[:, :, t * B:(t + 1) * B].rearrange("c p b -> p c b"))
            gq = []
            for q in range(4):
                gs = work.tile([P, U * B], F32, name=f"g{q}", tag=f"g{q}")
                nc.vector.tensor_tensor(
                    out=gs[:], in0=psg[q][:],
                    in1=xg_t[:, q * U:(q + 1) * U, :],
                    op=mybir.AluOpType.add)
                gq.append(gs)
            si = work.tile([P, U * B], F32, name="si", tag="si")
            nc.scalar.activation(si[:], gq[0][:], AF.Sigmoid)
            sf = work.tile([P, U * B], F32, name="sf", tag="sf")
            nc.scalar.activation(sf[:], gq[1][:], AF.Sigmoid)
            tg = work.tile([P, U * B], F32, name="tg", tag="tg")
            nc.scalar.activation(tg[:], gq[2][:], AF.Tanh)
            so = work.tile([P, U * B], F32, name="so", tag="so")
            nc.scalar.activation(so[:], gq[3][:], AF.Sigmoid)
            c_in = (c0_sb if c_prev is None else c_prev)
            c_new = state.tile([P, U * B], F32, name="c_new", tag="c_new")
            t1 = work.tile([P, U * B], F32, name="t1", tag="t1")
            nc.vector.tensor_mul(t1[:], sf[:], c_in[:])
            t2 = work.tile([P, U * B], F32, name="t2", tag="t2")
            nc.vector.tensor_mul(t2[:], si[:], tg[:])
            nc.vector.tensor_add(c_new[:], t1[:], t2[:])
            c_prev = c_new
            tc_t = work.tile([P, U * B], F32, name="tc_t", tag="tc_t")
            nc.scalar.activation(tc_t[:], c_new[:], AF.Tanh)
            if TP:
                h_bf = work.tile([P, B], HDT, name="h_bf", tag="h_bf")
                nc.vector.tensor_mul(h_bf[:], so[:], tc_t[:])
                nc.gpsimd.dma_start(cc_in[t][:], h_bf[:])
                nc.gpsimd.collective_compute(
                    "AllGather", mybir.AluOpType.bypass,
                    replica_groups=[list(range(NCORES))],
                    ins=[cc_in[t].opt()],
                    outs=[cc_out[t].opt()])
                for k in range(KH):
                    nc.sync.dma_start(hall[k][:, t * B:(t + 1) * B],
                                      cc_out[t][k * P:(k + 1) * P, :])
            else:
                for u in range(U):
                    nc.vector.tensor_mul(
                        hall[u][:, t * B:(t + 1) * B],
                        so[:, u * B:(u + 1) * B], tc_t[:, u * B:(u + 1) * B])

        # ---------------- attention (replicated over all 32 b) ----------------
        for b in range(B):
            ps_sc = ps_mm.tile([T, S], F32, name="ps_sc", tag="psA")
            for k in range(KH):
                et = stream.tile([P, S], HDT, name="et", tag="et")
                nc.sync.dma_start(et[:], encT[b, k * P:(k + 1) * P, :])
                hs = hall[k].rearrange("p (t b) -> p t b", b=B)
                nc.tensor.matmul(ps_sc[:], lhsT=hs[:, :, b],
                                 rhs=et[:],
                                 start=(k == 0), stop=(k == KH - 1))
            mx = work.tile([T, 1], F32, name="mx", tag="mx")
            nc.vector.tensor_reduce(mx[:], ps_sc[:], axis=mybir.AxisListType.X,
                                    op=mybir.AluOpType.max)
            nmx = work.tile([T, 1], F32, name="nmx", tag="nmx")
            nc.vector.tensor_scalar_mul(nmx[:], mx[:], -1.0)
            probs = work.tile([T, S], F32, name="probs", tag="probs")
            ssum = work.tile([T, 1], F32, name="ssum", tag="ssum")
            nc.scalar.activation(probs[:], ps_sc[:], AF.Exp, bias=nmx[:],
                                 accum_out=ssum[:])
            rec = work.tile([T, 1], F32, name="rec", tag="rec")
            nc.vector.reciprocal(rec[:], ssum[:])
            pn = work.tile([T, S], BF16, name="pn", tag="pn")
            nc.scalar.mul(pn[:], probs[:], rec[:])
            ps_at = ps_mm.tile([S, T], BF16, name="ps_at", tag="psB")
            nc.tensor.transpose(ps_at[:], pn[:], ident[:T, :T])
            attnT = work.tile([S, T], BF16, name="attnT", tag="attnT")
            nc.vector.tensor_copy(attnT[:], ps_at[:])
            for k in range(KH):
                ec = stream.tile([S, P], BF16, name="ec", tag="ec")
                nc.sync.dma_start(ec[:], enc[b, :, k * P:(k + 1) * P])
                ps_cx = ps_mm.tile([P, T], F32, name="ps_cx", tag="psA")
                nc.tensor.matmul(ps_cx[:], lhsT=ec[:],
                                 rhs=attnT[:], start=True, stop=True)
                nc.vector.tensor_copy(
                    ctxt[k].rearrange("p (t b) -> p t b", b=B)[:, :, b], ps_cx[:])

        # ---------------- decT = tanh(W_wT.T @ [h; ctx] + b_w) ----------------
        for mo in range(KH):
            wsh, wsc = [], []
            for k in range(KH):
                wh = stream.tile([P, P], HDT, name="wh", tag=f"wh{k}", bufs=2)
                nc.sync.dma_start(wh[:], w_wT_h[k * P:(k + 1) * P, mo * P:(mo + 1) * P])
                wsh.append(wh)
                wc = stream.tile([P, P], BF16, name="wc", tag=f"wc{k}", bufs=2)
                nc.sync.dma_start(wc[:], w_wT_c[k * P:(k + 1) * P, mo * P:(mo + 1) * P])
                wsc.append(wc)
            for quarter in range(4):
                n0, n1 = quarter * (R // 4), (quarter + 1) * (R // 4)
                ps_d = ps_mm.tile([P, R // 4], F32, name="ps_d", tag="psA")
                for k in range(2 * KH):
                    rhs = (hall[k] if k < KH else ctxt[k - KH])[:, n0:n1]
                    lhsT = wsh[k][:] if k < KH else wsc[k - KH][:]
                    nc.tensor.matmul(ps_d[:], lhsT=lhsT,
                                     rhs=rhs, start=(k == 0), stop=(k == 2 * KH - 1))
                nc.scalar.activation(dect[mo][:, n0:n1], ps_d[:], AF.Tanh,
                                     bias=bw_t[:, mo:mo + 1])

        # ---------------- vocab projection (V-sharded) ----------------
        for n in range(VN):
            wo_tiles = []
            for k in range(KH):
                wo = stream.tile([P, 500], BF16, name="wo", tag=f"wo{k}", bufs=2)
                nc.sync.dma_start(
                    wo[:], w_outT_s[k * P:(k + 1) * P, n * 500:(n + 1) * 500])
                wo_tiles.append(wo)
            for tg_i, (ta, tb) in enumerate(TGROUPS):
                m0, mw = ta * B, (tb - ta) * B
                ps_v = ps_mm.tile([P, 500], F32, name="ps_v", tag="psB")
                for k in range(KH):
                    nc.tensor.matmul(ps_v[:mw, :],
                                     lhsT=dect[k][:, m0:m0 + mw],
                                     rhs=wo_tiles[k][:],
                                     start=(k == 0), stop=False)
                nc.tensor.matmul(ps_v[:mw, :],
                                 lhsT=ones_t[0:1, :mw],
                                 rhs=bout_t[0:1, n * 500:(n + 1) * 500],
                                 start=False, stop=True)
                o_sb = work.tile([P, 500], F32, name="o_sb", tag="o_sb")
                nc.vector.tensor_copy(o_sb[:mw, :], ps_v[:mw, :])
                nc.sync.dma_start(
                    out_s[:, ta:tb, n * 500:(n + 1) * 500].transpose([1, 0, 2]),
                    o_sb[:mw, :])
    nc.compile()
    return nc


_CACHE = {}


def _get_graph():
    if "nc" not in _CACHE:
        _CACHE["nc"] = build_graph()
    return _CACHE["nc"]


def _prep(tgt_input, hidden_state, cell_state, encoder_outputs,
          embedding, W_ih, W_hh, b_ih, b_hh, W_w, b_w, W_out, b_out):
    """Host-side layout prep. Returns per-core input maps."""
    f32 = np.float32
    bf16 = ml_dtypes.bfloat16
    idx = np.asarray(tgt_input)[:, :-1].astype(np.int64)    # [B, T]
    emb = np.asarray(embedding, f32)[idx]                   # [B, T, E]
    x_embT = np.ascontiguousarray(emb.transpose(2, 1, 0).reshape(E, R)).astype(bf16)

    w_ihT = np.asarray(W_ih, f32).T                         # [E, G]
    w_hhT = np.asarray(W_hh, f32).T                         # [H, G]
    bias = (np.asarray(b_ih, f32) + np.asarray(b_hh, f32))  # [G]
    h0T = np.ascontiguousarray(np.asarray(hidden_state, f32)[0].T)
    if not TP:
        h0T = h0T.astype(bf16)
    c0T = np.ascontiguousarray(np.asarray(cell_state, f32)[0].T)  # [H, B]
    enc_b = np.asarray(encoder_outputs, f32).astype(bf16)   # [B, S, H]
    encT_b = np.ascontiguousarray(
        np.asarray(encoder_outputs, f32).transpose(0, 2, 1))
    if not TP:
        encT_b = encT_b.astype(bf16)
    w_wT_full = np.ascontiguousarray(np.asarray(W_w, f32).T)
    w_wT_h = w_wT_full[:H]
    if not TP:
        w_wT_h = w_wT_h.astype(bf16)
    w_wT_c = w_wT_full[H:].astype(bf16)
    b_w_sb = np.ascontiguousarray(np.asarray(b_w, f32).reshape(KH, P).T)
    w_outT = np.asarray(W_out, f32).T                       # [H, V]
    b_out_a = np.asarray(b_out, f32)

    in_maps = []
    for m in range(NCORES):
        # owned gate chunks: for quarter q, hidden chunks u -> global col block
        cols = []
        for q in range(4):
            for u in range(U):
                ch = m if TP else u
                j0 = q * H + ch * P
                cols.append(np.arange(j0, j0 + P))
        cols = np.concatenate(cols)                          # [CH*P]
        wih_s = np.ascontiguousarray(w_ihT[:, cols]).astype(bf16)
        whh_s = np.ascontiguousarray(w_hhT[:, cols])
        if not TP:
            whh_s = whh_s.astype(bf16)
        bias_sb = np.ascontiguousarray(bias[cols].reshape(CH, P).T)
        if TP:
            c0_s = np.ascontiguousarray(c0T[m * P:(m + 1) * P, :])
        else:
            c0_s = np.ascontiguousarray(
                c0T.reshape(KH, P, B).transpose(1, 0, 2).reshape(P, U * B))
        in_maps.append({
            "x_embT": x_embT,
            "w_ihT_s": wih_s,
            "w_hhT_s": whh_s,
            "bias_s": bias_sb,
            "h0T": h0T,
            "c0T_s": c0_s,
            "enc": enc_b,
            "encT": encT_b,
            "w_wT_h": w_wT_h,
            "w_wT_c": w_wT_c,
            "b_w_sb": b_w_sb,
            "w_outT_s": np.ascontiguousarray(
                w_outT[:, m * VL:(m + 1) * VL]).astype(bf16),
            "b_out_s": np.ascontiguousarray(
                b_out_a[m * VL:(m + 1) * VL]).reshape(1, VL).astype(bf16),
        })
    return in_maps


def kernel(**inputs) -> np.ndarray:
    nc = _get_graph()
    in_maps = _prep(**inputs)
    res = run_bass_kernel_spmd(nc, in_maps, list(range(NCORES)))
    outs = [res.results[m]["out_s"] for m in range(NCORES)]
    return np.concatenate(outs, axis=2)



# revision 25
# speedup vs baseline: 1.3358x; 1.3358x over previous
"""DecoderRNN Trainium2 kernel v2: 63-step LSTM + Luong attention + vocab projection.

Structure (8 NeuronCores, SPMD):
  - Recurrence TP8: gate dim sharded 8 ways (each core owns 128 hidden dims
    x 4 quarters, quarter order f,i,o,g); per-step AllGather of the h slice.
    h is exchanged as a bf16 (hi, lo) split pair so downstream consumers see
    ~f32 precision where needed (scores) while matmuls stay bf16.
  - Xg = W_ih x + bias precomputed into SBUF (stage A), added into the gate
    PSUM via an identity-stationary matmul (no extra vector op).
  - Attention + dect replicated over all 32 b (SPMD static-AP constraint),
    t-chunked so they pipeline into the recurrence AllGather gaps.
  - Vocab projection V-sharded (each core 4000 cols), interleaved as filler.
  - Output written bf16; host casts to f32.
"""

import numpy as np
import ml_dtypes
from collections import deque
from contextlib import ExitStack

import concourse.bass as bass
import concourse.bacc as bacc
import concourse.tile as tile
import concourse.mybir as mybir
from concourse import masks
from concourse.bass_utils import run_bass_kernel_spmd

F32 = mybir.dt.float32
BF16 = mybir.dt.bfloat16
AF = mybir.ActivationFunctionType
ALU = mybir.AluOpType

B, T, S = 32, 63, 64
V, E, H = 32000, 512, 1024
P = 128
NCORES = 8
R = T * B                     # 2016 rows, r = t*32 + b
VL = V // NCORES              # 4000
KH = H // P                   # 8
KE = E // P                   # 4
NQ = 4                        # gate quarters, order (f, i, o, g)
QPERM = [1, 0, 3, 2]          # ours q -> torch gate index (i,f,g,o)

CHUNKS = [(0, 32), (32, 63)]  # attention/dect/vocab t-chunks
NW = 4                        # stage A windows
RW = R // NW                  # 504
VN = 4                        # vocab col tiles per core
VW = VL // VN                 # 1000
# vocab row chunks: 15 x 128 + 96
RCHUNKS = [(i * 128, min((i + 1) * 128, R)) for i in range(16)]

GAP_NS = 5200                 # filler emission budget per step
HRING = 63                    # hall ring slots (t mod HRING)
XGR = R                       # Xg full (no ring)


def _tsplit(t0, t1):
    """Split [t0, t1) at HRING wrap boundaries -> list of (ta, tb)."""
    out = []
    a = t0
    while a < t1:
        b = min(t1, ((a // HRING) + 1) * HRING)
        out.append((a, b))
        a = b
    return out


def build_graph():
    nc = bacc.Bacc("TRN2", target_bir_lowering=False, debug=False,
                   num_devices=NCORES)

    def inp(name, shape, dtype):
        return nc.dram_tensor(name, list(shape), dtype, kind="ExternalInput").ap()

    x_embT = inp("x_embT", [E, R], BF16)
    wih_s = inp("wih_s", [E, NQ * P], BF16)
    whh_s = inp("whh_s", [H, NQ * P], BF16)
    bias_s = inp("bias_s", [P, NQ], F32)
    h0_pair = inp("h0_pair", [H, 2 * B], BF16)     # cols: hi(32) | lo(32)
    c0_s = inp("c0_s", [P, B], F32)
    enc = inp("enc", [B, S, H], BF16)
    encT = inp("encT", [B, H, S], BF16)
    w_wT_s = inp("w_wT_s", [2 * H, H], BF16)
    b_w_sb = inp("b_w_sb", [P, KH], F32)
    w_outT_s = inp("w_outT_s", [H, VL], BF16)
    bout_bc = inp("bout_bc", [P, VL], F32)
    out_s = nc.dram_tensor("out_s", [B, T, VL], BF16, kind="ExternalOutput").ap()
    dbg_hall = nc.dram_tensor("dbg_hall", [P, KH, HRING, 2, B], BF16,
                              kind="ExternalOutput").ap()
    dbg_dect = nc.dram_tensor("dbg_dect", [P, KH, R], BF16,
                              kind="ExternalOutput").ap()
    dbg_xg = nc.dram_tensor("dbg_xg", [P, NQ, XGR], BF16,
                            kind="ExternalOutput").ap()
    dbg_ctxt = nc.dram_tensor("dbg_ctxt", [P, KH, 32 * B], BF16,
                              kind="ExternalOutput").ap()
    dbg_ctxtA = nc.dram_tensor("dbg_ctxtA", [P, KH, 32 * B], BF16,
                               kind="ExternalOutput").ap()
    dbg_scA = nc.dram_tensor("dbg_scA", [S, S], F32,
                             kind="ExternalOutput").ap()
    dbg_g0 = nc.dram_tensor("dbg_g0", [P, NQ, B], F32,
                            kind="ExternalOutput").ap()
    dbg_h0t = nc.dram_tensor("dbg_h0t", [P, KH, 2, B], BF16,
                             kind="ExternalOutput").ap()

    with tile.TileContext(nc) as tc, ExitStack() as ctx:
        pool1 = ctx.enter_context(tc.tile_pool(name="pool1", bufs=1))
        stream = ctx.enter_context(tc.tile_pool(name="stream", bufs=1))
        work = ctx.enter_context(tc.tile_pool(name="work", bufs=2))
        state = ctx.enter_context(tc.tile_pool(name="state", bufs=2))
        ps_g = ctx.enter_context(tc.tile_pool(name="ps_g", bufs=1, space="PSUM"))
        ps_big = ctx.enter_context(tc.tile_pool(name="ps_big", bufs=2, space="PSUM"))
        ps_smf = ctx.enter_context(tc.tile_pool(name="ps_smf", bufs=2, space="PSUM"))
        ps_smb = ctx.enter_context(tc.tile_pool(name="ps_smb", bufs=1, space="PSUM"))
        dram = ctx.enter_context(tc.tile_pool(name="dram", bufs=1, space="DRAM"))

        # ---------------- resident tiles ----------------
        whh_t = pool1.tile([P, KH, NQ * P], BF16, name="whh_t")
        nc.sync.dma_start(whh_t[:], whh_s.rearrange("(k p) c -> p k c", p=P))
        wih_t = pool1.tile([P, KE, NQ * P], BF16, name="wih_t")
        nc.sync.dma_start(wih_t[:], wih_s.rearrange("(k p) c -> p k c", p=P))
        bias_t = pool1.tile([P, NQ], F32, name="bias_t")
        nc.sync.dma_start(bias_t[:], bias_s[:])
        bw_t = pool1.tile([P, KH], F32, name="bw_t")
        nc.sync.dma_start(bw_t[:], b_w_sb[:])
        ident = pool1.tile([P, P], BF16, name="ident")
        masks.make_identity(nc, ident[:])
        ident_f = pool1.tile([P, P], F32, name="ident_f")
        masks.make_identity(nc, ident_f[:])
        h0_t = pool1.tile([P, KH, 2, B], BF16, name="h0_t")
        nc.sync.dma_start(h0_t[:], h0_pair.rearrange("(k p) x -> p k x", p=P)
                          .rearrange("p k (pair b) -> p k pair b", pair=2))
        c0_t = pool1.tile([P, B], F32, name="c0_t")
        nc.sync.dma_start(c0_t[:], c0_s[:])

        # hall: [p, k, t, pair(hi/lo), b]
        hall = pool1.tile([P, KH, HRING, 2, B], BF16, name="hall")
        xg_t = pool1.tile([P, NQ, XGR], BF16, name="xg_t")
        dect = pool1.tile([P, KH, R], BF16, name="dect")
        # per-chunk ctx buffer [p, k, (t_local, b)]
        ctxt = pool1.tile([P, KH, 32 * B], BF16, name="ctxt")

        cc_in = [dram.tile([P, 2 * B], BF16, name=f"cc_in{t}") for t in range(T)]
        cc_out = [dram.tile([NCORES * P, 2 * B], BF16, name=f"cc_out{t}",
                            addr_space="Shared") for t in range(T)]

        # ---------------- unit emitters ----------------
        def stage_a(w):
            n0, n1 = w * RW, (w + 1) * RW
            xt = stream.tile([P, KE, RW], BF16, name="xa", tag="xa")
            nc.sync.dma_start(
                xt[:], x_embT[:, n0:n1].rearrange("(k p) r -> p k r", p=P))
            for q in range(NQ):
                ps = ps_big.tile([P, 1024], F32, name="ps_a", tag="bigps")
                for k in range(KE):
                    nc.tensor.matmul(ps[:, :RW],
                                     lhsT=wih_t[:, k, q * P:(q + 1) * P],
                                     rhs=xt[:, k, :],
                                     start=(k == 0), stop=(k == KE - 1))
                g0 = n0 % XGR
                nc.scalar.activation(xg_t[:, q, g0:g0 + RW], ps[:, :RW],
                                     AF.Identity, bias=bias_t[:, q:q + 1])

        def attn_unit(b, t0, t1):
            w = t1 - t0
            et = stream.tile([P, KH, S], BF16, name="et", tag="et")
            nc.sync.dma_start(et[:], encT[b].rearrange("(k p) s -> p k s", p=P))
            ps_f = ps_smf.tile([P, S], F32, name="ps_f", tag="asm_f")
            ps_sc = ps_f[:S, :]
            subs = _tsplit(t0, t1)
            for k in range(KH):
                for pair in range(2):
                    for ta, tb in subs:
                        nc.tensor.matmul(
                            ps_sc[:, ta - t0:tb - t0],
                            lhsT=et[:, k, :],
                            rhs=hall[:, k, ta % HRING:ta % HRING + (tb - ta),
                                     pair, b],
                            start=(k == 0 and pair == 0 and ta == t0),
                            stop=(k == KH - 1 and pair == 1
                                  and tb == t1))
            sc = work.tile([S, S], F32, name="sc", tag="sc")
            nc.vector.tensor_copy(sc[:, :w], ps_sc[:, :w])
            if b == 0 and t0 == 0:
                nc.sync.dma_start(dbg_scA, sc[:])
            ps_f2 = ps_smf.tile([P, S], F32, name="ps_f2", tag="asm_f")
            ps_tr = ps_f2[:, :S]
            nc.tensor.transpose(ps_tr[:w, :], sc[:, :w], ident_f[:S, :S])
            probs = work.tile([S, S], BF16, name="probs", tag="probs")
            ssum = work.tile([S, 1], F32, name="ssum", tag="ssum")
            nc.scalar.activation(probs[:w, :], ps_tr[:w, :], AF.Exp,
                                 accum_out=ssum[:w, :])
            rec = work.tile([S, 1], F32, name="rec", tag="rec")
            nc.vector.reciprocal(rec[:w, :], ssum[:w, :])
            pn = work.tile([S, S], BF16, name="pn", tag="pn")
            nc.scalar.mul(pn[:w, :], probs[:w, :], rec[:w, :])
            ps_b = ps_smb.tile([S, S], BF16, name="ps_at", tag="asm_b")
            ps_at = ps_b
            nc.tensor.transpose(ps_at[:, :w], pn[:w, :], ident[:w, :w])
            attnT = work.tile([S, S], BF16, name="attnT", tag="attnT")
            nc.vector.tensor_copy(attnT[:, :w], ps_at[:, :w])
            ec = stream.tile([S, KH, P], BF16, name="ec", tag="ec")
            nc.sync.dma_start(ec[:], enc[b].rearrange("s (k p) -> s k p", p=P))
            ctx_v = ctxt.rearrange("p k (t b) -> p k t b", b=B)
            for k in range(KH):
                ps_cxt = ps_smf.tile([P, S], F32, name="ps_cx", tag="asm_f")
                ps_cx = ps_cxt[:, :]
                nc.tensor.matmul(ps_cx[:, :w], lhsT=ec[:, k, :],
                                 rhs=attnT[:, :w], start=True, stop=True)
                nc.vector.tensor_copy(ctx_v[:, k, :w, b], ps_cx[:, :w])

        def dect_unit(mo, t0, ta, tb, ww_t):
            # (ta, tb): t-subrange of the chunk starting at t0
            cols = (tb - ta) * B
            ps_d = ps_big.tile([P, 1024], F32, name="ps_d", tag="bigps")
            for kk in range(2 * KH):
                if kk < KH:
                    sa = ta % HRING
                    rhs = hall[:, kk, sa:sa + (tb - ta), 0, :]
                else:
                    rhs = ctxt[:, kk - KH, (ta - t0) * B:(tb - t0) * B]
                nc.tensor.matmul(ps_d[:, :cols], lhsT=ww_t[:, kk, :],
                                 rhs=rhs, start=(kk == 0), stop=(kk == 2 * KH - 1))
            nc.scalar.activation(dect[:, mo, ta * B:tb * B],
                                 ps_d[:, :cols], AF.Tanh,
                                 bias=bw_t[:, mo:mo + 1])

        def ww_load(mo):
            ww_t = stream.tile([P, 2 * KH, P], BF16, name="ww", tag="ww")
            nc.sync.dma_start(
                ww_t[:], w_wT_s[:, mo * P:(mo + 1) * P]
                .rearrange("(k p) m -> p k m", p=P))
            return ww_t

        def wo_load(n):
            wo_t = stream.tile([P, KH, VW], BF16, name="wo", tag="wo")
            nc.sync.dma_start(
                wo_t[:], w_outT_s[:, n * VW:(n + 1) * VW]
                .rearrange("(k p) v -> p k v", p=P))
            bo_t = stream.tile([P, VW], F32, name="bo", tag="bo")
            nc.sync.dma_start(bo_t[:], bout_bc[:, n * VW:(n + 1) * VW])
            return wo_t, bo_t

        def vocab_unit(n, rc, wo_t, bo_t):
            r0, r1 = RCHUNKS[rc]
            rw = r1 - r0
            ps_v = ps_big.tile([P, 1024], F32, name="ps_v", tag="bigps")
            for k in range(KH):
                for hf in range(2):
                    nc.tensor.matmul(
                        ps_v[:rw, hf * 512:hf * 512 + 500],
                        lhsT=dect[:, k, r0:r1],
                        rhs=wo_t[:, k, hf * 500:(hf + 1) * 500],
                        start=(k == 0), stop=(k == KH - 1))
            o_sb = work.tile([P, VW], BF16, name="o_sb", tag="o_sb")
            for hf in range(2):
                nc.any.tensor_tensor(
                    out=o_sb[:rw, hf * 500:(hf + 1) * 500],
                    in0=ps_v[:rw, hf * 512:hf * 512 + 500],
                    in1=bo_t[:rw, hf * 500:(hf + 1) * 500], op=ALU.add)
            ta, tb = r0 // B, r1 // B
            nc.sync.dma_start(
                out_s[:, ta:tb, n * VW:(n + 1) * VW].transpose([1, 0, 2]),
                o_sb[:rw, :])

        # ---------------- filler queue ----------------
        # items: [est_ns, min_t, callable]
        queue = []

        def drain(budget_ns, now):
            spent = 0
            i = 0
            while i < len(queue) and spent < budget_ns:
                est, min_t, fn = queue[i]
                if min_t > now:
                    i += 1
                    continue
                queue.pop(i)
                fn()
                spent += est

        # stage A: window 0 up front, rest paced to the Xg ring
        stage_a(0)
        for w in range(1, NW):
            queue.append((3500, 0, lambda w=w: stage_a(w)))

        # ---------------- recurrence ----------------
        c_prev = c0_t
        for t in range(T):
            src = h0_t if t == 0 else hall[:, :, (t - 1) % HRING, :, :]
            psgt = ps_g.tile([P, NQ, B], F32, name="psg", tag="psg")
            psq = [psgt[:, q, :] for q in range(NQ)]
            # one start=True matmul seeds the whole bank with Xg (identity
            # stationary); everything else accumulates with start=False.
            # PSUM zeroing is bank-granular, so only ONE start per bank.
            g0 = t * B
            nc.tensor.matmul(psgt[:, :, :], lhsT=ident[:],
                             rhs=xg_t[:, :, g0:g0 + B],
                             start=True, stop=False, skip_group_check=True)
            # quarters: f(0), i(1), g(3) first, o(2) last
            for q in (0, 1, 3, 2):
                for k in range(KH):
                    for pair in range(2):
                        nc.tensor.matmul(
                            psq[q][:], lhsT=whh_t[:, k, q * P:(q + 1) * P],
                            rhs=src[:, k, pair, :],
                            start=False,
                            stop=(q == 2 and k == KH - 1 and pair == 1),
                            skip_group_check=True)

            if t == 0:
                g0sb = work.tile([P, NQ, B], F32, name="g0sb", tag="g0sb")
                nc.vector.tensor_copy(g0sb[:], psgt[:])
                nc.sync.dma_start(dbg_g0, g0sb[:])
            sf = work.tile([P, B], F32, name="sf", tag="sf")
            nc.scalar.activation(sf[:], psq[0][:], AF.Sigmoid)
            si = work.tile([P, B], F32, name="si", tag="si")
            nc.scalar.activation(si[:], psq[1][:], AF.Sigmoid)
            tg = work.tile([P, B], F32, name="tg", tag="tg")
            nc.scalar.activation(tg[:], psq[3][:], AF.Tanh)
            t1_ = work.tile([P, B], F32, name="t1", tag="t1")
            nc.vector.tensor_mul(t1_[:], sf[:], c_prev[:])
            t2_ = work.tile([P, B], F32, name="t2", tag="t2")
            nc.vector.tensor_mul(t2_[:], si[:], tg[:])
            c_new = state.tile([P, B], F32, name="c_new", tag="c_new")
            nc.vector.tensor_add(c_new[:], t1_[:], t2_[:])
            c_prev = c_new
            so = work.tile([P, B], F32, name="so", tag="so")
            nc.scalar.activation(so[:], psq[2][:], AF.Sigmoid)
            tc_t = work.tile([P, B], F32, name="tc", tag="tc")
            nc.scalar.activation(tc_t[:], c_new[:], AF.Tanh)
            h_f32 = work.tile([P, B], F32, name="h_f32", tag="h_f32")
            nc.vector.tensor_mul(h_f32[:], so[:], tc_t[:])
            h_cc = work.tile([P, 2, B], BF16, name="h_cc", tag="h_cc")
            nc.any.tensor_mul(h_cc[:, 0, :], so[:], tc_t[:])
            nc.vector.tensor_tensor(out=h_cc[:, 1, :], in0=h_f32[:],
                                    in1=h_cc[:, 0, :], op=ALU.subtract)
            nc.gpsimd.dma_start(cc_in[t][:], h_cc.rearrange("p x b -> p (x b)"))
            nc.gpsimd.collective_compute(
                "AllGather", ALU.bypass,
                replica_groups=[list(range(NCORES))],
                ins=[cc_in[t].opt()],
                outs=[cc_out[t].opt()])
            nc.sync.dma_start(
                hall[:, :, t % HRING, :, :],
                cc_out[t].rearrange("(k p) x -> p k x", p=P)
                .rearrange("p k (pair b) -> p k pair b", pair=2))

            # chunk completions -> queue filler
            for ci, (t0, t1) in enumerate(CHUNKS):
                if t == t1 - 1:
                    for b in range(B):
                        queue.append((900, 0, lambda b=b, t0=t0, t1=t1:
                                      attn_unit(b, t0, t1)))
                    if ci == 0:
                        queue.append((0, 0, lambda:
                                      nc.sync.dma_start(dbg_ctxtA, ctxt[:])))
                    # sub-ranges: wrap-aware, each <= 16 t
                    dsubs = []
                    for sa, sb in _tsplit(t0, t1):
                        while sb - sa > 16:
                            dsubs.append((sa, sa + 16))
                            sa += 16
                        dsubs.append((sa, sb))
                    for mo in range(KH):
                        def dect_pair(mo=mo, t0=t0, dsubs=tuple(dsubs)):
                            ww_t = ww_load(mo)
                            for sa, sb in dsubs:
                                dect_unit(mo, t0, sa, sb, ww_t)
                        queue.append((7000, 0, dect_pair))
                    # vocab units, grouped by n with a load unit
                    lo = 8 * ci
                    hi = 8 * (ci + 1) if ci == 0 else len(RCHUNKS)
                    for n in range(VN):
                        holder = {}
                        def wo_unit(n=n, holder=holder):
                            holder["wo"] = wo_load(n)
                        queue.append((500, 0, wo_unit))
                        for rc in range(lo, hi):
                            queue.append((3600, 0, lambda n=n, rc=rc,
                                          holder=holder:
                                          vocab_unit(n, rc, *holder["wo"])))

            drain(GAP_NS, t)

        # ---------------- epilogue: drain everything ----------------
        drain(1 << 60, 1 << 30)
        nc.sync.dma_start(dbg_hall, hall[:])
        nc.sync.dma_start(dbg_dect, dect[:])
        nc.sync.dma_start(dbg_xg, xg_t[:])
        nc.sync.dma_start(dbg_ctxt, ctxt[:])
        nc.sync.dma_start(dbg_h0t, h0_t[:])

    nc.compile()
    return nc


_CACHE = {}


def _get_graph():
    if "nc" not in _CACHE:
        _CACHE["nc"] = build_graph()
    return _CACHE["nc"]


def _prep(tgt_input, hidden_state, cell_state, encoder_outputs,
          embedding, W_ih, W_hh, b_ih, b_hh, W_w, b_w, W_out, b_out):
    f32 = np.float32
    bf16 = ml_dtypes.bfloat16
    idx = np.asarray(tgt_input)[:, :-1].astype(np.int64)        # [B, T]
    emb = np.asarray(embedding, f32)[idx]                       # [B, T, E]
    x_embT = np.ascontiguousarray(
        emb.transpose(2, 1, 0).reshape(E, R)).astype(bf16)

    w_ihT = np.asarray(W_ih, f32).T                             # [E, 4H]
    w_hhT = np.asarray(W_hh, f32).T                             # [H, 4H]
    bias = (np.asarray(b_ih, f32) + np.asarray(b_hh, f32))      # [4H]

    h0 = np.asarray(hidden_state, f32)[0]                       # [B, H]
    h0T = np.ascontiguousarray(h0.T)                            # [H, B]
    h0_hi = h0T.astype(bf16)
    h0_lo = (h0T - h0_hi.astype(f32)).astype(bf16)
    h0_pair = np.concatenate([h0_hi, h0_lo], axis=1)            # [H, 2B]

    c0T = np.ascontiguousarray(np.asarray(cell_state, f32)[0].T)  # [H, B]
    enc_b = np.asarray(encoder_outputs, f32).astype(bf16)       # [B, S, H]
    encT_b = np.ascontiguousarray(
        np.asarray(encoder_outputs, f32).transpose(0, 2, 1)).astype(bf16)
    w_wT = np.ascontiguousarray(np.asarray(W_w, f32).T).astype(bf16)  # [2H, H]
    b_w_sb = np.ascontiguousarray(np.asarray(b_w, f32).reshape(KH, P).T)
    w_outT = np.asarray(W_out, f32).T                           # [H, V]
    b_out_a = np.asarray(b_out, f32)

    in_maps = []
    for m in range(NCORES):
        cols = []
        for q in range(NQ):
            j0 = QPERM[q] * H + m * P
            cols.append(np.arange(j0, j0 + P))
        cols = np.concatenate(cols)
        wih_s = np.ascontiguousarray(w_ihT[:, cols]).astype(bf16)
        whh_s = np.ascontiguousarray(w_hhT[:, cols]).astype(bf16)
        bias_sb = np.ascontiguousarray(bias[cols].reshape(NQ, P).T)
        c0_sl = np.ascontiguousarray(c0T[m * P:(m + 1) * P, :])
        in_maps.append({
            "x_embT": x_embT,
            "wih_s": wih_s,
            "whh_s": whh_s,
            "bias_s": bias_sb,
            "h0_pair": h0_pair,
            "c0_s": c0_sl,
            "enc": enc_b,
            "encT": encT_b,
            "w_wT_s": w_wT,
            "b_w_sb": b_w_sb,
            "w_outT_s": np.ascontiguousarray(
                w_outT[:, m * VL:(m + 1) * VL]).astype(bf16),
            "bout_bc": np.ascontiguousarray(np.broadcast_to(
                b_out_a[m * VL:(m + 1) * VL], (P, VL))).astype(f32),
        })
    return in_maps


def kernel(**inputs) -> np.ndarray:
    nc = _get_graph()
    in_maps = _prep(**inputs)
    res = run_bass_kernel_spmd(nc, in_maps, list(range(NCORES)))
    outs = [np.asarray(res.results[m]["out_s"]).astype(np.float32)
            for m in range(NCORES)]
    return np.concatenate(outs, axis=2)
